# revision 17
# baseline (speedup 1.0000x reference)
"""Trainium2 Bass kernel for nn_MicroTransformerLayer.

Sharding: 8 cores = 4 sequences x 2 half-sequence shards. Each core receives
the full sequence's x (bf16, transposed, permuted so its own 1024 tokens sit
at context positions 1024:2047), recomputes the full-sequence down-projection
(needed for K/V), and runs attention + FF + up-projection for its own 1024
tokens. SPMD-uniform program; per-core differences enter only through data
(a +0/-10000 additive gate bias on the other-half attention scores).

Performance structure (vs the 823us starting point):
- bf16 activations/weights/IO (halves DMA; DVE fast modes).
- Token-major x streaming: down-projection, RMSNorm and QKV run per
  512-token context chunk so attention overlaps the x DMA stream instead
  of serializing behind it.
- All RMSNorms computed as rsqrt on the vector engine (shift-seed + Newton,
  ~5e-6 rel err): the scalar engine's activation table stays pinned to Exp
  for the whole kernel (softmax + silu), zero mid-kernel table loads.
- Softmax denominators via reciprocal_approx_fast; diagonal attention tiles
  narrowed to the causal q-window; DMA issue spread across 3 engine queues.
"""

import os
import sys

for _p in ("/opt/trn_rl_repo", "/root/.axon_site/_ro/trn_rl_repo"):
    if os.path.isdir(_p) and _p not in sys.path:
        sys.path.append(_p)

import numpy as np
import ml_dtypes

import concourse.bass as bass
import concourse.mybir as mybir
import concourse.tile as tile
from concourse import bacc
from concourse.bass_utils import run_bass_kernel_spmd

F32 = mybir.dt.float32
BF16 = mybir.dt.bfloat16
U32 = mybir.dt.uint32
I32 = mybir.dt.int32
AF = mybir.ActivationFunctionType
OP = mybir.AluOpType
BF16NP = ml_dtypes.bfloat16

BIG, SMALL, HEADS, HD, FF = 4096, 256, 4, 64, 512
B, T = 4, 2048
SEQ, OWN = 2048, 1024
P, CH = 128, 512
KT_BIG = BIG // P             # 32
CTX_TILES = SEQ // P          # 16
EPS = 1.1920929e-07
GATE_OFF = -10000.0
MAGIC = float(0x5F3759DF)
N_CORES = 8


def _emit_rsqrt(nc, wk, out_ap, ps_in, scale, bias):
    """out = 1/sqrt(ps_in*scale + bias), DVE only (shift seed + 2 Newton).

    ps_in: [P, CH] f32 (PSUM ok). out_ap: [P, CH] f32 SBUF destination.
    """
    vm = wk.tile([P, CH], F32, tag="vm")
    nc.vector.tensor_scalar(vm[:], ps_in, scale, bias, OP.mult, OP.add)
    sh = wk.tile([P, CH], F32, tag="sh")
    nc.vector.tensor_scalar(sh[:].bitcast(U32), vm[:].bitcast(U32),
                            1, None, OP.logical_shift_right)
    hf = wk.tile([P, CH], F32, tag="hf")
    nc.vector.tensor_copy(hf[:], sh[:].bitcast(I32))
    nc.vector.tensor_scalar(hf[:], hf[:], -1.0, MAGIC, OP.mult, OP.add)
    nc.vector.tensor_copy(sh[:].bitcast(I32), hf[:])
    av = wk.tile([P, CH], F32, tag="av")
    for _ in range(2):
        nc.vector.tensor_mul(av[:], sh[:], sh[:])
        nc.vector.tensor_mul(av[:], vm[:], av[:])
        nc.vector.tensor_scalar(av[:], av[:], -0.5, 1.5, OP.mult, OP.add)
        nc.vector.tensor_mul(sh[:], sh[:], av[:])
    nc.vector.tensor_copy(out_ap, sh[:])


def _emit(nc, tc, d):
    with (
        tc.tile_pool(name="persist", bufs=1) as pp,
        tc.tile_pool(name="xin", bufs=2) as xin,
        tc.tile_pool(name="hbp", bufs=2) as hbp,
        tc.tile_pool(name="hnp", bufs=2) as hnp,
        tc.tile_pool(name="prp", bufs=2) as prp,
        tc.tile_pool(name="wk", bufs=2) as wk,
        tc.tile_pool(name="outp", bufs=2) as outp,
        tc.tile_pool(name="psA", bufs=4, space="PSUM") as psA,
        tc.tile_pool(name="psS", bufs=2, space="PSUM") as psS,
    ):
        # ---- persistent SBUF tiles; stage-A-critical DMAs first ----
        w_dd = pp.tile([P, KT_BIG, SMALL], BF16, tag="wdd")
        nc.sync.dma_start(w_dd[:, 0:4, :], d["wd"][:, 0:4, :])
        nc.sync.dma_start(w_dd[:, 4:16, :], d["wd"][:, 4:16, :])
        nc.scalar.dma_start(w_dd[:, 16:32, :], d["wd"][:, 16:32, :])
        cb_s = pp.tile([P, 2], F32, tag="cbias")   # col0: gate bias, col1: eps
        nc.scalar.dma_start(cb_s[:], d["cbias"])
        w_qkv = pp.tile([P, 2, 3 * SMALL], BF16, tag="wqkv")
        w_o = pp.tile([P, 2, SMALL], BF16, tag="wo")
        w_gu = pp.tile([P, 2, 2 * FF], BF16, tag="wgu")
        w_dff = pp.tile([P, 4, SMALL], BF16, tag="wdff")
        ones_s = pp.tile([P, P], BF16, tag="ones")
        tril_s = pp.tile([P, P], BF16, tag="tril")

        hT = pp.tile([P, 2, OWN], F32, tag="hT")          # residual (own half)
        kT = pp.tile([P, 2, SEQ], BF16, tag="kT")
        qT = pp.tile([P, 2, OWN], BF16, tag="qT")
        vo = pp.tile([P, CTX_TILES, 4 * (HD + 1)], BF16, tag="vo")
        aoT = pp.tile([P, 2, OWN], BF16, tag="aoT")

        # =============== STAGE A: per-512-token-chunk pipeline ================
        # x chunk DMA -> down-proj (weight resident) -> rsqrt norm -> QKV
        for c in range(4):
            cs = bass.ds(c * CH, CH)
            ph = psS.tile([P, 2 * CH], F32, tag="s")       # h accum (m0|m1)
            for half in range(2):
                xt = xin.tile([P, 16, CH], BF16, tag="xt")
                xeng = nc.gpsimd if (2 * c + half) % 2 == 0 else nc.sync
                xeng.dma_start(xt[:], d["x"][c, :, bass.ts(half, 16), :])
                if c == 0 and half == 0:
                    nc.scalar.dma_start(w_qkv[:], d["wqkv"])
                    nc.scalar.dma_start(ones_s[:], d["ones"])
                elif c == 0 and half == 1:
                    nc.scalar.dma_start(tril_s[:], d["tril"])
                    nc.scalar.dma_start(
                        vo[:].rearrange("p t (h x) -> p t h x", x=HD + 1)
                        [:, :, :, HD : HD + 1],
                        d["vones"].rearrange("p (t h) -> p t h", h=4)
                        [:, :, :, None],
                    )
                elif c == 1 and half == 0:
                    nc.scalar.dma_start(w_o[:], d["wo"])
                    nc.scalar.dma_start(w_dff[:], d["wdff"])
                elif c == 1 and half == 1:
                    nc.scalar.dma_start(w_gu[:], d["wgu"])
                for k16 in range(16):
                    k = 16 * half + k16
                    for m in range(2):
                        nc.tensor.matmul(ph[:, bass.ts(m, CH)],
                                         w_dd[:, k, bass.ts(m, P)],
                                         xt[:, k16, :],
                                         start=(k == 0), stop=(k == KT_BIG - 1))
            # evacuate h (bf16 everywhere + f32 residual for own chunks)
            hb = hbp.tile([P, 2, CH], BF16, tag="hb")
            nc.vector.tensor_copy(hb[:, 0, :], ph[:, 0:CH])
            nc.scalar.copy(hb[:, 1, :], ph[:, CH : 2 * CH])
            if c >= 2:
                hs = bass.ds((c - 2) * CH, CH)
                nc.vector.tensor_copy(hT[:, 0, hs], ph[:, 0:CH])
                nc.scalar.copy(hT[:, 1, hs], ph[:, CH : 2 * CH])
            hsq = wk.tile([P, 2 * CH], BF16, tag="hsq")
            nc.vector.tensor_mul(hsq[:], hb[:], hb[:])
            pss = psA.tile([P, CH], F32, tag="a")
            for m in range(2):
                nc.tensor.matmul(pss[:], ones_s[:], hsq[:, bass.ts(m, CH)],
                                 start=(m == 0), stop=(m == 1))
            rinv = wk.tile([P, CH], F32, tag="rinv")
            _emit_rsqrt(nc, wk, rinv[:], pss[:], 1.0 / SMALL, EPS)
            hn = hnp.tile([P, 2, CH], BF16, tag="hn")
            for m in range(2):
                nc.vector.tensor_mul(hn[:, m, :], hb[:, m, :], rinv[:])
            # K (all chunks) / Q (own chunks)
            for m in range(2):
                pk = psA.tile([P, CH], F32, tag="a")
                for kt in range(2):
                    nc.tensor.matmul(pk[:], w_qkv[:, kt, bass.ds(SMALL + m * P, P)],
                                     hn[:, kt, :], start=(kt == 0), stop=(kt == 1))
                nc.scalar.copy(kT[:, m, cs], pk[:])
                if c >= 2:
                    pq = psA.tile([P, CH], F32, tag="a")
                    for kt in range(2):
                        nc.tensor.matmul(pq[:], w_qkv[:, kt, bass.ds(m * P, P)],
                                         hn[:, kt, :], start=(kt == 0), stop=(kt == 1))
                    nc.vector.tensor_copy(qT[:, m, bass.ds((c - 2) * CH, CH)], pq[:])
            # V token-major with per-head ones columns
            for tt in range(4):
                ct = 4 * c + tt
                pv = psA.tile([P, SMALL], F32, tag="a")
                for kt in range(2):
                    nc.tensor.matmul(pv[:], hn[:, kt, bass.ts(tt, P)],
                                     w_qkv[:, kt, bass.ds(2 * SMALL, SMALL)],
                                     start=(kt == 0), stop=(kt == 1))
                nc.vector.tensor_copy(
                    vo[:, ct, :].rearrange("p (h x) -> p h x", x=HD + 1)[:, :, 0:HD],
                    pv[:].rearrange("p (h x) -> p h x", x=HD),
                )

        # preload W_up.T (after the x stream)
        w_up = pp.tile([P, 2, BIG], BF16, tag="wup")
        for _q in range(4):
            nc.gpsimd.dma_start(w_up[:, :, bass.ts(_q, BIG // 4)],
                                d["wup"][:, :, bass.ts(_q, BIG // 4)])

        # =============== STAGES B+C per own 512-token chunk ===================
        for c in range(2):
            qs = bass.ds(c * CH, CH)
            vis = 12 + 4 * c
            diag0 = 8 + 4 * c
            # ---- attention ----
            for ft in range(2):
                po = [psA.tile([HD + 1, CH], F32, tag="a", name=f"po{_h}")
                      for _h in range(2)]
                for kt in range(vis):
                    di = kt - diag0
                    q0 = di * P if di >= 0 else 0      # causal-narrowed q window
                    w = CH - q0
                    ps_s = psS.tile([P, 2 * CH], F32, tag="s")
                    for hh in range(2):
                        nc.tensor.matmul(
                            ps_s[:, bass.ds(hh * CH + q0, w)],
                            kT[HD * hh : HD * hh + HD, ft, bass.ts(kt, P)],
                            qT[HD * hh : HD * hh + HD, ft, bass.ds(c * CH + q0, w)],
                            start=True, stop=True,
                        )
                    pr = prp.tile([P, 2 * CH], BF16, tag="pr")
                    if kt < 8:
                        nc.scalar.activation(pr[:], ps_s[:], AF.Exp,
                                             bias=cb_s[:, 0:1], scale=0.125)
                    elif di < 0:
                        nc.scalar.activation(pr[:], ps_s[:], AF.Exp, scale=0.125)
                    else:
                        for hh in range(2):
                            sl = bass.ds(hh * CH + q0, w)
                            nc.scalar.activation(pr[:, sl], ps_s[:, sl], AF.Exp,
                                                 scale=0.125)
                        for hh in range(2):
                            sl = bass.ds(hh * CH + q0, P)
                            nc.vector.tensor_mul(pr[:, sl], pr[:, sl], tril_s[:])
                    for hh in range(2):
                        h4 = 2 * ft + hh
                        nc.tensor.matmul(
                            po[hh][:, bass.ds(q0, w)],
                            vo[:, kt, bass.ts(h4, HD + 1)],
                            pr[:, bass.ds(hh * CH + q0, w)],
                            start=(kt == 0), stop=(kt == vis - 1),
                        )
                # normalize: broadcast Z, reciprocal into SBUF, multiply
                for hh in range(2):
                    zb = wk.tile([P, CH], BF16, tag="zb")
                    nc.vector.tensor_copy(zb[HD : HD + 1, :],
                                          po[hh][HD : HD + 1, :])
                    pb = psS.tile([P, CH], F32, tag="s")
                    nc.tensor.matmul(pb[0:HD, :], ones_s[HD : HD + 1, 0:HD],
                                     zb[HD : HD + 1, :], start=True, stop=True)
                    rb = wk.tile([P, CH], F32, tag="rb")
                    nc.vector.reciprocal_approx_fast(out=rb[0:HD, :],
                                                     in_=pb[0:HD, :])
                    nc.vector.tensor_mul(aoT[HD * hh : HD * hh + HD, ft, qs],
                                         po[hh][0:HD, :], rb[0:HD, :])

            # ---- o-proj + norm2 + FF + up-proj ----
            h2 = wk.tile([P, 2, CH], F32, tag="h2")
            for m in range(2):
                pp_ = psA.tile([P, CH], F32, tag="a")
                for kt in range(2):
                    nc.tensor.matmul(pp_[:], w_o[:, kt, bass.ts(m, P)],
                                     aoT[:, kt, qs], start=(kt == 0), stop=(kt == 1))
                nc.vector.tensor_add(h2[:, m, :], pp_[:], hT[:, m, qs])
            h2sq = wk.tile([P, 2 * CH], BF16, tag="hsq")
            nc.vector.tensor_mul(h2sq[:], h2[:], h2[:])
            pss2 = psA.tile([P, CH], F32, tag="a")
            for m in range(2):
                nc.tensor.matmul(pss2[:], ones_s[:], h2sq[:, bass.ts(m, CH)],
                                 start=(m == 0), stop=(m == 1))
            rinv2 = wk.tile([P, CH], F32, tag="rinv")
            _emit_rsqrt(nc, wk, rinv2[:], pss2[:], 1.0 / SMALL, EPS)
            hn2 = hnp.tile([P, 2, CH], BF16, tag="hn")
            for m in range(2):
                nc.vector.tensor_mul(hn2[:, m, :], h2[:, m, :], rinv2[:])
            # FF with exp-based silu + fast reciprocal
            fT = wk.tile([P, 4, CH], BF16, tag="fT")
            for g in range(4):
                pg = psA.tile([P, CH], F32, tag="a")
                for kt in range(2):
                    nc.tensor.matmul(pg[:], w_gu[:, kt, bass.ts(g, P)],
                                     hn2[:, kt, :], start=(kt == 0), stop=(kt == 1))
                pu = psA.tile([P, CH], F32, tag="a")
                for kt in range(2):
                    nc.tensor.matmul(pu[:], w_gu[:, kt, bass.ds(FF + g * P, P)],
                                     hn2[:, kt, :], start=(kt == 0), stop=(kt == 1))
                ex = wk.tile([P, CH], F32, tag="ex")
                nc.scalar.activation(ex[:], pg[:], AF.Exp, scale=-1.0)
                exp1 = wk.tile([P, CH], F32, tag="exp1")
                nc.vector.tensor_scalar_add(exp1[:], ex[:], 1.0)
                rc = wk.tile([P, CH], F32, tag="ex")
                nc.vector.reciprocal_approx_fast(out=rc[:], in_=exp1[:])
                xs = wk.tile([P, CH], F32, tag="exp1")
                nc.vector.tensor_mul(xs[:], pg[:], rc[:])
                nc.vector.tensor_mul(fT[:, g, :], xs[:], pu[:])
            # ff down + residual
            h3 = wk.tile([P, 2, CH], BF16, tag="h3")
            for m in range(2):
                pf = psA.tile([P, CH], F32, tag="a")
                for kt in range(4):
                    nc.tensor.matmul(pf[:], w_dff[:, kt, bass.ts(m, P)],
                                     fT[:, kt, :], start=(kt == 0), stop=(kt == 3))
                nc.vector.tensor_add(h3[:, m, :], pf[:], h2[:, m, :])
            # up-projection: m-tile pairs share a 2-bank psum tile so the
            # PSUM->SBUF evacuation runs as wide [128,1024] copies
            for mb in range(KT_BIG // 4):
                yt = outp.tile([P, 4, CH], BF16, tag="yt")
                for kp in range(2):
                    py = psS.tile([P, 2 * CH], F32, tag="s")
                    for kk in range(2):
                        m = 4 * mb + 2 * kp + kk
                        for kt in range(2):
                            nc.tensor.matmul(py[:, bass.ts(kk, CH)],
                                             w_up[:, kt, bass.ts(m, P)],
                                             h3[:, kt, :],
                                             start=(kt == 0), stop=(kt == 1))
                    if kp == 0:
                        nc.vector.tensor_copy(yt[:, 0:2, :], py[:])
                    else:
                        nc.scalar.copy(yt[:, 2:4, :], py[:])
                yeng = nc.gpsimd if mb % 2 == 0 else nc.sync
                yeng.dma_start(d["y"][:, bass.ts(mb, 4), qs], yt[:])


def _build():
    nc = bacc.Bacc("TRN2", target_bir_lowering=False, debug=False,
                   num_devices=N_CORES)
    d = {}
    d["x"] = nc.dram_tensor("x", [4, P, KT_BIG, CH], BF16, kind="ExternalInput").ap()
    d["wd"] = nc.dram_tensor("wd", [P, KT_BIG, SMALL], BF16, kind="ExternalInput").ap()
    d["wqkv"] = nc.dram_tensor("wqkv", [P, 2, 3 * SMALL], BF16, kind="ExternalInput").ap()
    d["wo"] = nc.dram_tensor("wo", [P, 2, SMALL], BF16, kind="ExternalInput").ap()
    d["wgu"] = nc.dram_tensor("wgu", [P, 2, 2 * FF], BF16, kind="ExternalInput").ap()
    d["wdff"] = nc.dram_tensor("wdff", [P, 4, SMALL], BF16, kind="ExternalInput").ap()
    d["wup"] = nc.dram_tensor("wup", [P, 2, BIG], BF16, kind="ExternalInput").ap()
    d["ones"] = nc.dram_tensor("ones", [P, P], BF16, kind="ExternalInput").ap()
    d["tril"] = nc.dram_tensor("tril", [P, P], BF16, kind="ExternalInput").ap()
    d["cbias"] = nc.dram_tensor("cbias", [P, 2], F32, kind="ExternalInput").ap()
    d["vones"] = nc.dram_tensor("vones", [P, 64], BF16, kind="ExternalInput").ap()
    d["y"] = nc.dram_tensor("y", [P, KT_BIG, OWN], BF16, kind="ExternalOutput").ap()
    with tile.TileContext(nc) as tc:
        _emit(nc, tc, d)
    nc.compile()
    return nc


_NC_CACHE = None


def _get_nc():
    global _NC_CACHE
    if _NC_CACHE is None:
        _NC_CACHE = _build()
    return _NC_CACHE


def _ki_major(a, n_tiles):
    """[n_tiles*128, m] -> [128, n_tiles, m] (partition-major packing)."""
    m = a.shape[1]
    return np.ascontiguousarray(
        a.reshape(n_tiles, P, m).transpose(1, 0, 2).astype(BF16NP))


def make_in_maps(x, W_down, W_up, W_qkv, W_o, W_gate, W_upff, W_downff, g1, g2):
    f32 = np.float32
    shared = {
        "wd": _ki_major(np.asarray(W_down, f32).T, KT_BIG),
        "wqkv": _ki_major(np.ascontiguousarray(
            (np.asarray(W_qkv, f32) * np.asarray(g1, f32)[None, :]).T), 2),
        "wo": _ki_major(np.asarray(W_o, f32).T, 2),
        "wgu": _ki_major(np.ascontiguousarray(
            (np.concatenate([np.asarray(W_gate, f32), np.asarray(W_upff, f32)], axis=0)
             * np.asarray(g2, f32)[None, :]).T), 2),
        "wdff": _ki_major(np.asarray(W_downff, f32).T, 4),
        "wup": _ki_major(np.asarray(W_up, f32).T, 2),
        "ones": np.ones((P, P), BF16NP),
        "vones": np.ones((P, 64), BF16NP),
    }
    # tril[p, j] = 1 if p <= j (inclusive lower-triangular for diagonal tiles)
    pi = np.arange(P)[:, None]
    jj = np.arange(P)[None, :]
    shared["tril"] = (pi <= jj).astype(BF16NP)
    in_maps = []
    for b in range(B):
        for j in range(2):
            other = x[b, (1 - j) * OWN : (2 - j) * OWN]
            own = x[b, j * OWN : (j + 1) * OWN]
            xp = np.concatenate([other, own], axis=0)          # [SEQ, BIG]
            xT = np.ascontiguousarray(xp.T).astype(BF16NP)     # [BIG, SEQ]
            # [4 chunks, 128, 32 ktiles, 512]
            xc = xT.reshape(KT_BIG, P, 4, CH).transpose(2, 1, 0, 3)
            m = dict(shared)
            m["x"] = np.ascontiguousarray(xc)
            cb = np.empty((P, 2), f32)
            cb[:, 0] = 0.0 if j == 1 else GATE_OFF
            cb[:, 1] = EPS
            m["cbias"] = cb
            in_maps.append(m)
    return in_maps


def assemble(results):
    y = np.empty((B, T, BIG), np.float32)
    for b in range(B):
        for j in range(2):
            yT = results[2 * b + j]["y"]                  # [128, 32, 1024] bf16
            yf = yT.astype(np.float32).transpose(1, 0, 2).reshape(BIG, OWN)
            y[b, j * OWN : (j + 1) * OWN] = yf.T
    return y


def kernel(x, W_down, W_up, W_qkv, W_o, W_gate, W_upff, W_downff, g1, g2):
    nc = _get_nc()
    in_maps = make_in_maps(x, W_down, W_up, W_qkv, W_o, W_gate, W_upff,
                           W_downff, g1, g2)
    res = run_bass_kernel_spmd(nc, in_maps, core_ids=list(range(N_CORES)))
    return assemble(res.results)


# revision 20
# speedup vs baseline: 4.1405x; 4.1405x over previous
"""Trainium2 Bass kernel for nn_MicroTransformerLayer.

Sharding: 8 cores = 4 sequences x 2 half-sequence shards. Each core receives
the full sequence's x (bf16, transposed, permuted so its own 1024 tokens sit
at context positions 1024:2047), recomputes the full-sequence down-projection
(needed for K/V), and runs attention + FF + up-projection for its own 1024
tokens. SPMD-uniform program; per-core differences enter only through data
(a +0/-10000 additive gate bias on the other-half attention scores).

Performance structure (vs the 823us starting point):
- bf16 activations/weights/IO (halves DMA; DVE fast modes).
- Token-major x streaming: down-projection, RMSNorm and QKV run per
  512-token context chunk so attention overlaps the x DMA stream instead
  of serializing behind it.
- All RMSNorms computed as rsqrt on the vector engine (shift-seed + Newton,
  ~5e-6 rel err): the scalar engine's activation table stays pinned to Exp
  for the whole kernel (softmax + silu), zero mid-kernel table loads.
- Softmax denominators via reciprocal_approx_fast; diagonal attention tiles
  narrowed to the causal q-window; DMA issue spread across 3 engine queues.
"""

import os
import sys

for _p in ("/opt/trn_rl_repo", "/root/.axon_site/_ro/trn_rl_repo"):
    if os.path.isdir(_p) and _p not in sys.path:
        sys.path.append(_p)

import numpy as np
import ml_dtypes

import concourse.bass as bass
import concourse.mybir as mybir
import concourse.tile as tile
from concourse import bacc
from concourse.bass_utils import run_bass_kernel_spmd

F32 = mybir.dt.float32
BF16 = mybir.dt.bfloat16
U32 = mybir.dt.uint32
I32 = mybir.dt.int32
AF = mybir.ActivationFunctionType
OP = mybir.AluOpType
BF16NP = ml_dtypes.bfloat16

BIG, SMALL, HEADS, HD, FF = 4096, 256, 4, 64, 512
B, T = 4, 2048
SEQ, OWN = 2048, 1024
P, CH = 128, 512
KT_BIG = BIG // P             # 32
CTX_TILES = SEQ // P          # 16
EPS = 1.1920929e-07
GATE_OFF = -10000.0
MAGIC = float(0x5F3759DF)
N_CORES = 8


def _emit_rsqrt(nc, wk, out_ap, ps_in, scale, bias):
    """out = 1/sqrt(ps_in*scale + bias), DVE only (shift seed + 2 Newton).

    ps_in: [P, CH] f32 (PSUM ok). out_ap: [P, CH] f32 SBUF destination.
    """
    vm = wk.tile([P, CH], F32, tag="vm")
    nc.vector.tensor_scalar(vm[:], ps_in, scale, bias, OP.mult, OP.add)
    sh = wk.tile([P, CH], F32, tag="sh")
    nc.vector.tensor_scalar(sh[:].bitcast(U32), vm[:].bitcast(U32),
                            1, None, OP.logical_shift_right)
    hf = wk.tile([P, CH], F32, tag="hf")
    nc.vector.tensor_copy(hf[:], sh[:].bitcast(I32))
    nc.vector.tensor_scalar(hf[:], hf[:], -1.0, MAGIC, OP.mult, OP.add)
    nc.vector.tensor_copy(sh[:].bitcast(I32), hf[:])
    av = wk.tile([P, CH], F32, tag="av")
    for _ in range(2):
        nc.vector.tensor_mul(av[:], sh[:], sh[:])
        nc.vector.tensor_mul(av[:], vm[:], av[:])
        nc.vector.tensor_scalar(av[:], av[:], -0.5, 1.5, OP.mult, OP.add)
        nc.vector.tensor_mul(sh[:], sh[:], av[:])
    nc.vector.tensor_copy(out_ap, sh[:])


def _emit(nc, tc, d):
    with (
        tc.tile_pool(name="persist", bufs=1) as pp,
        tc.tile_pool(name="xin", bufs=3) as xin,
        tc.tile_pool(name="hbp", bufs=2) as hbp,
        tc.tile_pool(name="hnp", bufs=2) as hnp,
        tc.tile_pool(name="prp", bufs=2) as prp,
        tc.tile_pool(name="wk", bufs=2) as wk,
        tc.tile_pool(name="outp", bufs=2) as outp,
        tc.tile_pool(name="psA", bufs=4, space="PSUM") as psA,
        tc.tile_pool(name="psS", bufs=2, space="PSUM") as psS,
    ):
        # ---- persistent SBUF tiles; stage-A-critical DMAs first ----
        w_dd = pp.tile([P, KT_BIG, SMALL], BF16, tag="wdd")
        nc.sync.dma_start(w_dd[:, 0:4, :], d["wd"][:, 0:4, :])
        nc.scalar.dma_start(w_dd[:, 4:16, :], d["wd"][:, 4:16, :])
        nc.scalar.dma_start(w_dd[:, 16:32, :], d["wd"][:, 16:32, :])
        cb_s = pp.tile([P, 2], F32, tag="cbias")   # col0: gate bias, col1: eps
        nc.scalar.dma_start(cb_s[:], d["cbias"])
        w_qkv = pp.tile([P, 2, 3 * SMALL], BF16, tag="wqkv")
        nc.scalar.dma_start(w_qkv[:], d["wqkv"])
        w_o = pp.tile([P, 2, SMALL], BF16, tag="wo")
        nc.scalar.dma_start(w_o[:], d["wo"])
        w_gu = pp.tile([P, 2, 2 * FF], BF16, tag="wgu")
        nc.scalar.dma_start(w_gu[:], d["wgu"])
        w_dff = pp.tile([P, 4, SMALL], BF16, tag="wdff")
        nc.scalar.dma_start(w_dff[:], d["wdff"])
        ones_s = pp.tile([P, P], BF16, tag="ones")
        nc.scalar.dma_start(ones_s[:], d["ones"])
        tril_s = pp.tile([P, P], BF16, tag="tril")
        nc.scalar.dma_start(tril_s[:], d["tril"])

        hT = pp.tile([P, 2, OWN], F32, tag="hT")          # residual (own half)
        kT = pp.tile([P, 2, SEQ], BF16, tag="kT")
        qT = pp.tile([P, 2, OWN], BF16, tag="qT")
        vo = pp.tile([P, CTX_TILES, 4 * (HD + 1)], BF16, tag="vo")
        aoT = pp.tile([P, 2, OWN], BF16, tag="aoT")
        nc.scalar.dma_start(
            vo[:].rearrange("p t (h x) -> p t h x", x=HD + 1)
            [:, :, :, HD : HD + 1],
            d["vones"].rearrange("p (t h) -> p t h", h=4)[:, :, :, None],
        )
        w_up = pp.tile([P, 2, BIG], BF16, tag="wup")

        # =============== STAGE A pipeline helpers =============================
        phs = {}      # chunk -> psum h accumulator

        def emit_xdma(c):
            tiles = []
            for q in range(4):
                xt = xin.tile([P, 8, CH], BF16, tag="xt")
                xeng = nc.gpsimd if q % 2 == 0 else nc.sync
                xeng.dma_start(xt[:], d["x"][c, :, bass.ts(q, 8), :])
                tiles.append(xt)
            return tiles

        def emit_down(c, xtiles, qrange):
            if c not in phs:
                phs[c] = psS.tile([P, 2 * CH], F32, tag="s", name=f"ph{c}")
            ph = phs[c]
            for q in qrange:
                for k8 in range(8):
                    k = 8 * q + k8
                    for m in range(2):
                        nc.tensor.matmul(ph[:, bass.ts(m, CH)],
                                         w_dd[:, k, bass.ts(m, P)],
                                         xtiles[q][:, k8, :],
                                         start=(k == 0), stop=(k == KT_BIG - 1))

        hbs, rinvs, hns = {}, {}, {}

        def emit_norm(c):
            ph = phs[c]
            hb = hbp.tile([P, 2, CH], BF16, tag="hb", name=f"hb{c}")
            nc.vector.tensor_copy(hb[:, 0, :], ph[:, 0:CH])
            nc.scalar.copy(hb[:, 1, :], ph[:, CH : 2 * CH])
            if c >= 2:
                hs = bass.ds((c - 2) * CH, CH)
                nc.vector.tensor_copy(hT[:, 0, hs], ph[:, 0:CH])
                nc.scalar.copy(hT[:, 1, hs], ph[:, CH : 2 * CH])
            del phs[c]
            hsq = wk.tile([P, 2 * CH], BF16, tag="hsq")
            nc.vector.tensor_mul(hsq[:], hb[:], hb[:])
            pss = psA.tile([P, CH], F32, tag="a")
            for m in range(2):
                nc.tensor.matmul(pss[:], ones_s[:], hsq[:, bass.ts(m, CH)],
                                 start=(m == 0), stop=(m == 1))
            rinv = wk.tile([P, CH], F32, tag="rinv")
            _emit_rsqrt(nc, wk, rinv[:], pss[:], 1.0 / SMALL, EPS)
            hbs[c], rinvs[c] = hb, rinv

        def emit_qkv(c):
            cs = bass.ds(c * CH, CH)
            hn = hnp.tile([P, 2, CH], BF16, tag="hn")
            for m in range(2):
                nc.vector.tensor_mul(hn[:, m, :], hbs[c][:, m, :], rinvs[c][:])
            for m in range(2):
                pk = psA.tile([P, CH], F32, tag="a")
                for kt in range(2):
                    nc.tensor.matmul(pk[:], w_qkv[:, kt, bass.ds(SMALL + m * P, P)],
                                     hn[:, kt, :], start=(kt == 0), stop=(kt == 1))
                nc.scalar.copy(kT[:, m, cs], pk[:])
                if c >= 2:
                    pq = psA.tile([P, CH], F32, tag="a")
                    for kt in range(2):
                        nc.tensor.matmul(pq[:], w_qkv[:, kt, bass.ds(m * P, P)],
                                         hn[:, kt, :], start=(kt == 0), stop=(kt == 1))
                    nc.vector.tensor_copy(qT[:, m, bass.ds((c - 2) * CH, CH)], pq[:])
            for tt in range(4):
                ct = 4 * c + tt
                pv = psA.tile([P, SMALL], F32, tag="a")
                for kt in range(2):
                    nc.tensor.matmul(pv[:], hn[:, kt, bass.ts(tt, P)],
                                     w_qkv[:, kt, bass.ds(2 * SMALL, SMALL)],
                                     start=(kt == 0), stop=(kt == 1))
                nc.vector.tensor_copy(
                    vo[:, ct, :].rearrange("p (h x) -> p h x", x=HD + 1)[:, :, 0:HD],
                    pv[:].rearrange("p (h x) -> p h x", x=HD),
                )

        def emit_attn(c, ft):
            qs = bass.ds(c * CH, CH)
            vis = 12 + 4 * c
            diag0 = 8 + 4 * c
            po = [psA.tile([HD + 1, CH], F32, tag="a", name=f"po{_h}")
                  for _h in range(2)]
            for kt in range(vis):
                di = kt - diag0
                q0 = di * P if di >= 0 else 0      # causal-narrowed q window
                w = CH - q0
                ps_s = psS.tile([P, 2 * CH], F32, tag="s")
                for hh in range(2):
                    nc.tensor.matmul(
                        ps_s[:, bass.ds(hh * CH + q0, w)],
                        kT[HD * hh : HD * hh + HD, ft, bass.ts(kt, P)],
                        qT[HD * hh : HD * hh + HD, ft, bass.ds(c * CH + q0, w)],
                        start=True, stop=True,
                    )
                pr = prp.tile([P, 2 * CH], BF16, tag="pr")
                if kt < 8:
                    nc.scalar.activation(pr[:], ps_s[:], AF.Exp,
                                         bias=cb_s[:, 0:1], scale=0.125)
                elif di < 0:
                    nc.scalar.activation(pr[:], ps_s[:], AF.Exp, scale=0.125)
                else:
                    for hh in range(2):
                        sl = bass.ds(hh * CH + q0, w)
                        nc.scalar.activation(pr[:, sl], ps_s[:, sl], AF.Exp,
                                             scale=0.125)
                    for hh in range(2):
                        sl = bass.ds(hh * CH + q0, P)
                        nc.vector.tensor_mul(pr[:, sl], pr[:, sl], tril_s[:])
                for hh in range(2):
                    h4 = 2 * ft + hh
                    nc.tensor.matmul(
                        po[hh][:, bass.ds(q0, w)],
                        vo[:, kt, bass.ts(h4, HD + 1)],
                        pr[:, bass.ds(hh * CH + q0, w)],
                        start=(kt == 0), stop=(kt == vis - 1),
                    )
            # normalize: broadcast Z, reciprocal into SBUF, multiply
            for hh in range(2):
                zb = wk.tile([P, CH], BF16, tag="zb")
                nc.vector.tensor_copy(zb[HD : HD + 1, :],
                                      po[hh][HD : HD + 1, :])
                pb = psA.tile([P, CH], F32, tag="a")
                nc.tensor.matmul(pb[0:HD, :], ones_s[HD : HD + 1, 0:HD],
                                 zb[HD : HD + 1, :], start=True, stop=True)
                rb = wk.tile([P, CH], F32, tag="rb")
                nc.vector.reciprocal_approx_fast(out=rb[0:HD, :],
                                                 in_=pb[0:HD, :])
                nc.vector.tensor_mul(aoT[HD * hh : HD * hh + HD, ft, qs],
                                     po[hh][0:HD, :], rb[0:HD, :])

        h2s, rinv2s = {}, {}

        def emit_oproj_norm2(c):
            qs = bass.ds(c * CH, CH)
            h2 = wk.tile([P, 2, CH], F32, tag="h2", name=f"h2_{c}")
            for m in range(2):
                pp_ = psA.tile([P, CH], F32, tag="a")
                for kt in range(2):
                    nc.tensor.matmul(pp_[:], w_o[:, kt, bass.ts(m, P)],
                                     aoT[:, kt, qs], start=(kt == 0), stop=(kt == 1))
                nc.vector.tensor_add(h2[:, m, :], pp_[:], hT[:, m, qs])
            h2sq = wk.tile([P, 2 * CH], BF16, tag="hsq")
            nc.vector.tensor_mul(h2sq[:], h2[:], h2[:])
            pss2 = psA.tile([P, CH], F32, tag="a")
            for m in range(2):
                nc.tensor.matmul(pss2[:], ones_s[:], h2sq[:, bass.ts(m, CH)],
                                 start=(m == 0), stop=(m == 1))
            rinv2 = wk.tile([P, CH], F32, tag="rinv", name=f"rinv2_{c}")
            _emit_rsqrt(nc, wk, rinv2[:], pss2[:], 1.0 / SMALL, EPS)
            h2s[c], rinv2s[c] = h2, rinv2

        def emit_ffup(c):
            qs = bass.ds(c * CH, CH)
            h2 = h2s[c]
            hn2 = hnp.tile([P, 2, CH], BF16, tag="hn")
            for m in range(2):
                nc.vector.tensor_mul(hn2[:, m, :], h2[:, m, :], rinv2s[c][:])
            # FF with exp-based silu + fast reciprocal
            fT = wk.tile([P, 4, CH], BF16, tag="fT")
            for g in range(4):
                pg = psA.tile([P, CH], F32, tag="a")
                for kt in range(2):
                    nc.tensor.matmul(pg[:], w_gu[:, kt, bass.ts(g, P)],
                                     hn2[:, kt, :], start=(kt == 0), stop=(kt == 1))
                pu = psA.tile([P, CH], F32, tag="a")
                for kt in range(2):
                    nc.tensor.matmul(pu[:], w_gu[:, kt, bass.ds(FF + g * P, P)],
                                     hn2[:, kt, :], start=(kt == 0), stop=(kt == 1))
                ex = wk.tile([P, CH], F32, tag="ex")
                nc.scalar.activation(ex[:], pg[:], AF.Exp, scale=-1.0)
                exp1 = wk.tile([P, CH], F32, tag="exp1")
                nc.vector.tensor_scalar_add(exp1[:], ex[:], 1.0)
                rc = wk.tile([P, CH], F32, tag="ex")
                nc.vector.reciprocal_approx_fast(out=rc[:], in_=exp1[:])
                xs = wk.tile([P, CH], F32, tag="exp1")
                nc.vector.tensor_mul(xs[:], pg[:], rc[:])
                nc.vector.tensor_mul(fT[:, g, :], xs[:], pu[:])
            # ff down + residual
            h3 = wk.tile([P, 2, CH], BF16, tag="h3")
            for m in range(2):
                pf = psA.tile([P, CH], F32, tag="a")
                for kt in range(4):
                    nc.tensor.matmul(pf[:], w_dff[:, kt, bass.ts(m, P)],
                                     fT[:, kt, :], start=(kt == 0), stop=(kt == 3))
                nc.vector.tensor_add(h3[:, m, :], pf[:], h2[:, m, :])
            # up-projection: m-tile pairs share a 2-bank psum tile so the
            # PSUM->SBUF evacuation runs as wide [128,1024] copies
            for mb in range(KT_BIG // 4):
                yt = outp.tile([P, 4, CH], BF16, tag="yt")
                for kp in range(2):
                    py = psS.tile([P, 2 * CH], F32, tag="s")
                    for kk in range(2):
                        m = 4 * mb + 2 * kp + kk
                        for kt in range(2):
                            nc.tensor.matmul(py[:, bass.ts(kk, CH)],
                                             w_up[:, kt, bass.ts(m, P)],
                                             h3[:, kt, :],
                                             start=(kt == 0), stop=(kt == 1))
                    if kp == 0:
                        nc.vector.tensor_copy(yt[:, 0:2, :], py[:])
                    else:
                        nc.scalar.copy(yt[:, 2:4, :], py[:])
                yeng = nc.gpsimd if mb % 2 == 0 else nc.sync
                yeng.dma_start(d["y"][:, bass.ts(mb, 4), qs], yt[:])

        # =============== orchestration (static software pipeline) =============
        # stage A: stagger so each chunk's rsqrt chain and QKV hide behind the
        # next chunk's down-projection; x DMAs always issue as early as possible
        xq = {c: emit_xdma(c) for c in range(2)}
        emit_down(0, xq[0], range(4))
        xq[2] = emit_xdma(2)
        emit_down(1, xq[1], range(4))
        emit_norm(0)
        xq[3] = emit_xdma(3)
        emit_down(2, xq[2], range(4))
        emit_norm(1)
        emit_qkv(0)
        emit_down(3, xq[3], range(2))          # first half of last chunk
        emit_norm(2)
        emit_qkv(1)
        emit_qkv(2)
        emit_attn(0, 0)                        # fills the last x-DMA wait
        emit_down(3, xq[3], range(2, 4))
        emit_norm(3)
        for _q in range(4):
            nc.gpsimd.dma_start(w_up[:, :, bass.ts(_q, BIG // 4)],
                                d["wup"][:, :, bass.ts(_q, BIG // 4)])
        emit_qkv(3)
        emit_attn(0, 1)
        # stages B/C interleaved: each rsqrt/normalize chain overlaps the next
        # block's matmuls
        emit_attn(1, 0)
        emit_oproj_norm2(0)
        emit_attn(1, 1)
        emit_ffup(0)
        emit_oproj_norm2(1)
        emit_ffup(1)



def _build():
    nc = bacc.Bacc("TRN2", target_bir_lowering=False, debug=False,
                   num_devices=N_CORES)
    d = {}
    d["x"] = nc.dram_tensor("x", [4, P, KT_BIG, CH], BF16, kind="ExternalInput").ap()
    d["wd"] = nc.dram_tensor("wd", [P, KT_BIG, SMALL], BF16, kind="ExternalInput").ap()
    d["wqkv"] = nc.dram_tensor("wqkv", [P, 2, 3 * SMALL], BF16, kind="ExternalInput").ap()
    d["wo"] = nc.dram_tensor("wo", [P, 2, SMALL], BF16, kind="ExternalInput").ap()
    d["wgu"] = nc.dram_tensor("wgu", [P, 2, 2 * FF], BF16, kind="ExternalInput").ap()
    d["wdff"] = nc.dram_tensor("wdff", [P, 4, SMALL], BF16, kind="ExternalInput").ap()
    d["wup"] = nc.dram_tensor("wup", [P, 2, BIG], BF16, kind="ExternalInput").ap()
    d["ones"] = nc.dram_tensor("ones", [P, P], BF16, kind="ExternalInput").ap()
    d["tril"] = nc.dram_tensor("tril", [P, P], BF16, kind="ExternalInput").ap()
    d["cbias"] = nc.dram_tensor("cbias", [P, 2], F32, kind="ExternalInput").ap()
    d["vones"] = nc.dram_tensor("vones", [P, 64], BF16, kind="ExternalInput").ap()
    d["y"] = nc.dram_tensor("y", [P, KT_BIG, OWN], BF16, kind="ExternalOutput").ap()
    with tile.TileContext(nc) as tc:
        _emit(nc, tc, d)
    nc.compile()
    return nc


_NC_CACHE = None


def _get_nc():
    global _NC_CACHE
    if _NC_CACHE is None:
        _NC_CACHE = _build()
    return _NC_CACHE


def _ki_major(a, n_tiles):
    """[n_tiles*128, m] -> [128, n_tiles, m] (partition-major packing)."""
    m = a.shape[1]
    return np.ascontiguousarray(
        a.reshape(n_tiles, P, m).transpose(1, 0, 2).astype(BF16NP))


def make_in_maps(x, W_down, W_up, W_qkv, W_o, W_gate, W_upff, W_downff, g1, g2):
    f32 = np.float32
    shared = {
        "wd": _ki_major(np.asarray(W_down, f32).T, KT_BIG),
        "wqkv": _ki_major(np.ascontiguousarray(
            (np.asarray(W_qkv, f32) * np.asarray(g1, f32)[None, :]).T), 2),
        "wo": _ki_major(np.asarray(W_o, f32).T, 2),
        "wgu": _ki_major(np.ascontiguousarray(
            (np.concatenate([np.asarray(W_gate, f32), np.asarray(W_upff, f32)], axis=0)
             * np.asarray(g2, f32)[None, :]).T), 2),
        "wdff": _ki_major(np.asarray(W_downff, f32).T, 4),
        "wup": _ki_major(np.asarray(W_up, f32).T, 2),
        "ones": np.ones((P, P), BF16NP),
        "vones": np.ones((P, 64), BF16NP),
    }
    # tril[p, j] = 1 if p <= j (inclusive lower-triangular for diagonal tiles)
    pi = np.arange(P)[:, None]
    jj = np.arange(P)[None, :]
    shared["tril"] = (pi <= jj).astype(BF16NP)
    in_maps = []
    for b in range(B):
        for j in range(2):
            other = x[b, (1 - j) * OWN : (2 - j) * OWN]
            own = x[b, j * OWN : (j + 1) * OWN]
            xp = np.concatenate([other, own], axis=0)          # [SEQ, BIG]
            xT = np.ascontiguousarray(xp.T).astype(BF16NP)     # [BIG, SEQ]
            # [4 chunks, 128, 32 ktiles, 512]
            xc = xT.reshape(KT_BIG, P, 4, CH).transpose(2, 1, 0, 3)
            m = dict(shared)
            m["x"] = np.ascontiguousarray(xc)
            cb = np.empty((P, 2), f32)
            cb[:, 0] = 0.0 if j == 1 else GATE_OFF
            cb[:, 1] = EPS
            m["cbias"] = cb
            in_maps.append(m)
    return in_maps


def assemble(results):
    y = np.empty((B, T, BIG), np.float32)
    for b in range(B):
        for j in range(2):
            yT = results[2 * b + j]["y"]                  # [128, 32, 1024] bf16
            yf = yT.astype(np.float32).transpose(1, 0, 2).reshape(BIG, OWN)
            y[b, j * OWN : (j + 1) * OWN] = yf.T
    return y


def kernel(x, W_down, W_up, W_qkv, W_o, W_gate, W_upff, W_downff, g1, g2):
    nc = _get_nc()
    in_maps = make_in_maps(x, W_down, W_up, W_qkv, W_o, W_gate, W_upff,
                           W_downff, g1, g2)
    res = run_bass_kernel_spmd(nc, in_maps, core_ids=list(range(N_CORES)))
    return assemble(res.results)
